# revision 9
# baseline (speedup 1.0000x reference)
"""Single-head causal attention (B=4, T=4096, C=1024, H=64) on 8 trn2 cores.

Sharding: each core owns one (batch b = i//2, query-interleave j = i%2) pair.
Queries of core (b, j) are the 8 interleaved 256-row chunks (2s+j)*256 of
batch b, which balances causal-attention work across the two cores of a
batch.  Every core receives the full (transposed, bf16) x of its batch and
computes K/V for all 4096 rows; Q only for its own 2048 rows.

v2 layout highlights:
  - K/V generated per 512-row tile rt with weights [Wk|Wv] for even rt and
    [Wv|Wk] for odd rt, so K^T of even blocks lives at SBUF partitions 0-63
    and K^T of odd blocks at partitions 64-127.  Score matmuls (contraction
    K=64 over the head dim) are then ROW-PACKED two-at-a-time onto the PE
    array with tile_position (0,0)/(64,0) — doubling score throughput and
    hiding LDWEIGHTS.
  - Q is computed with duplicated weights [Wq|Wq] so q^T exists at both
    partition halves (rhs for both row-tiles).
  - scores are computed transposed [k, q]; softmax runs without
    max-subtraction; the denominator comes from an all-ones 65th column on
    V-natural; causal masking multiplies exp outputs on the boundary tiles
    only.
  - output stored bf16 in the osb layout; host de-tiles and casts.
"""

import sys

sys.path.insert(0, "/opt/trn_rl_repo")

from contextlib import ExitStack

import ml_dtypes
import numpy as np

import concourse.bass as bass
import concourse.mybir as mybir
import concourse.tile as tile_mod
from concourse.bass_utils import run_bass_kernel_spmd
from concourse.tile import TileContext
from concourse.vector_clock import ScopedClock

# ---------------------------------------------------------------------------
# Workaround: this walrus accepts only ONE sync wait per Drain instruction.
# Split the TileContext exit-drain's waits across multiple drains.
# ---------------------------------------------------------------------------


def _patched_drain_and_barrier(self, tick_clock, wait_clock):
    drain_inst = self.nc.sync.drain()
    wait_clock.add_sem_waits(
        drain_inst.ins, ScopedClock({None: tick_clock.global_clock})
    )
    si = drain_inst.ins.sync_info
    waits = list(si.on_wait or []) if si is not None else []
    if len(waits) > 1:
        si.on_wait = waits[:1]
        for w in waits[1:]:
            d = self.nc.sync.drain()
            dsi = d.ins.sync_info
            if dsi is None:
                d.ins.sync_info = mybir.SyncInfo(on_wait=[w], on_update=[])
            else:
                dsi.on_wait = [w]

    self.nc.all_engine_barrier()
    assert self.sems is not None
    popped = self.nc._tile_sem_poison_stack.pop()
    assert popped is self._sem_poison
    self.nc.clear_and_free_semaphores(list(self.sems.allocated().values()))
    self.nc.all_engine_barrier()


tile_mod.TileContext._drain_and_barrier = _patched_drain_and_barrier


def _split_sync_waits(nc):
    """Rewrite any instruction carrying >1 sync wait into a chain of
    single-wait nops (same engine, inserted just before it)."""
    f = nc.m.functions[0]
    created = []  # names of nops we created (they get appended to cur_bb)

    plans = []  # (block, list of (inst_name, extra_waits))
    for blk in f.blocks:
        insts = list(blk.instructions)
        plan = {}
        for inst in insts:
            si = inst.sync_info
            waits = list(si.on_wait or []) if si is not None else []
            if len(waits) > 1:
                plan[inst.name] = waits[:-1]
                si.on_wait = waits[-1:]
        if plan:
            plans.append((blk, plan))

    nop_map = {}  # inst_name -> list of nop instructions
    for blk, plan in plans:
        for iname, extra in plan.items():
            nops = []
            for w in extra:
                eng_type = nc.inst_map[iname].engine
                bi = nc.engines[eng_type].nop(nofuse=True)
                bi.ins.sync_info = mybir.SyncInfo(on_wait=[w], on_update=[])
                created.append(bi.ins.name)
                nops.append(bi.ins)
            nop_map[iname] = nops

    created_set = set(created)
    for blk in f.blocks:
        newl = []
        for inst in blk.instructions:
            if inst.name in created_set:
                continue  # remove from wherever the builder appended it
            if inst.name in nop_map:
                newl.extend(nop_map[inst.name])
            newl.append(inst)
        blk.instructions = newl

# ---------------------------------------------------------------------------

B, T, C, H = 4, 4096, 1024, 64
NCORES = 8
TQ = T // 2          # queries per core
NSLOT = 8            # 256-query slots per core
QS = TQ // NSLOT     # 256
CB = C // 128        # 8 contraction chunks
NRT = T // 512       # 8 row tiles for k/v generation
BF16 = mybir.dt.bfloat16
F32 = mybir.dt.float32
EXPF = mybir.ActivationFunctionType.Exp

_prog_cache = {}


def _build_program():
    nc = bass.Bass("TRN2", target_bir_lowering=False, debug=False,
                   num_devices=NCORES)

    # xt/xqt are host-retiled to [tile, p, c, col] so each per-tile DMA reads
    # fully-sequential DRAM and lands contiguous per SBUF partition.
    xt_d = nc.dram_tensor("xt", [NRT, 128, CB, 512], BF16, kind="ExternalInput")
    xqt_d = nc.dram_tensor("xqt", [4, 128, CB, 512], BF16, kind="ExternalInput")
    # weights pretiled to [p, c, w] (partition-contiguous DMA)
    wkva_d = nc.dram_tensor("wkva", [128, CB, 128], BF16, kind="ExternalInput")
    wkvb_d = nc.dram_tensor("wkvb", [128, CB, 128], BF16, kind="ExternalInput")
    wq2_d = nc.dram_tensor("wq2", [128, CB, 128], BF16, kind="ExternalInput")
    mask_d = nc.dram_tensor("mask", [128, 4, 768], BF16, kind="ExternalInput")
    id_d = nc.dram_tensor("ident", [65, 65], BF16, kind="ExternalInput")
    y_d = nc.dram_tensor("y", [4, 128, 4, H], BF16, kind="ExternalOutput")

    with TileContext(nc) as tc, ExitStack() as ctx:
        const_p = ctx.enter_context(tc.tile_pool(name="const", bufs=1))
        xt_p = ctx.enter_context(tc.tile_pool(name="xt", bufs=1))
        big_p = ctx.enter_context(tc.tile_pool(name="big", bufs=1))
        exp_p = ctx.enter_context(tc.tile_pool(name="exp", bufs=6))
        out_p = ctx.enter_context(tc.tile_pool(name="outs", bufs=4))
        pm_p = ctx.enter_context(tc.tile_pool(name="pmisc", bufs=2, space="PSUM"))
        ps_p = ctx.enter_context(tc.tile_pool(name="pscore", bufs=2, space="PSUM"))
        po_p = ctx.enter_context(tc.tile_pool(name="pout", bufs=2, space="PSUM"))

        # persistent sbuf tensors
        xt_sb = xt_p.tile([128, NRT, CB, 512], BF16, tag="xt")
        xqt_sb = xt_p.tile([128, 4, CB, 512], BF16, tag="xqt")
        kvt_sb = big_p.tile([128, NRT, 512], BF16, tag="kvt")
        vte_sb = big_p.tile([64, 4, 512], BF16, tag="vte")
        qt2_sb = big_p.tile([128, TQ], BF16, tag="qt2")
        vnat_sb = big_p.tile([128, T // 128, H + 1], BF16, tag="vnat")
        warm_sb = big_p.tile([128, 512], BF16, tag="warm")
        nc.gpsimd.memset(warm_sb[:], 0.25)
        nc.gpsimd.memset(vnat_sb[:], 1.0)

        # PE warm-up: ~5us of dummy matmuls so the HAM clock-gate opens
        # (K=8/8) before the first real matmul, instead of ramping through
        # the kv generation.  Output bank is overwritten by real work later.
        warm_ps = pm_p.tile([128, 512], F32, tag="pm", name="warmps")
        for w in range(14):
            nc.tensor.matmul(warm_ps[:], lhsT=warm_sb[:, 0:128],
                             rhs=warm_sb[:], start=True, stop=True)

        # constants (tiles declared up front; DMAs emitted in load order below)
        wkva_sb = const_p.tile([128, CB, 128], BF16, tag="wkva")
        wkvb_sb = const_p.tile([128, CB, 128], BF16, tag="wkvb")
        wq2_sb = const_p.tile([128, CB, 128], BF16, tag="wq2")
        mask_sb = const_p.tile([128, 4, 768], BF16, tag="mask")
        id_sb = const_p.tile([65, 65], BF16, tag="ident")

        def load_xt(rt, split=False):
            if split:
                # two c-half DMAs so kv_mm can start after the first half
                nc.scalar.dma_start(out=xt_sb[:, rt, 0:4, :],
                                    in_=xt_d.ap()[rt, :, 0:4, :])
                nc.sync.dma_start(out=xt_sb[:, rt, 4:8, :],
                                  in_=xt_d.ap()[rt, :, 4:8, :])
            else:
                nc.sync.dma_start(out=xt_sb[:, rt, :, :], in_=xt_d.ap()[rt])

        def load_xqt(qt):
            nc.sync.dma_start(out=xqt_sb[:, qt, :, :], in_=xqt_d.ap()[qt])

        def kv_mm_tasks(rt):
            """kv generation for tile rt as a list of emission thunks."""
            w_sb = wkva_sb if rt % 2 == 0 else wkvb_sb
            box = {}

            def chunk(c0, rt=rt, w_sb=w_sb, box=box):
                if c0 == 0:
                    box["pkv"] = pm_p.tile([128, 512], F32, tag="pm",
                                           name=f"pkv{rt}")
                pkv = box["pkv"]
                for c in range(c0, c0 + 2):
                    nc.tensor.matmul(pkv[:], lhsT=w_sb[:, c, :],
                                     rhs=xt_sb[:, rt, c, :],
                                     start=(c == 0), stop=(c == CB - 1))
                if c0 == CB - 2:
                    nc.vector.tensor_copy(kvt_sb[:, rt, :], pkv[:])
                    if rt % 2 == 0:
                        nc.vector.tensor_copy(vte_sb[:, rt // 2, :],
                                              pkv[64:128, :])
            return [lambda c0=c0: chunk(c0) for c0 in range(0, CB, 2)]

        def kv_tr_tasks(rt):
            def tr(t, rt=rt):
                kb = rt * 4 + t
                src = (vte_sb[:, rt // 2, t * 128:(t + 1) * 128] if rt % 2 == 0
                       else kvt_sb[0:64, rt, t * 128:(t + 1) * 128])
                pt = pm_p.tile([128, 64], BF16, tag="pm")
                nc.tensor.transpose(pt[:], src, id_sb[0:64, 0:64])
                nc.vector.tensor_copy(vnat_sb[:, kb, 0:H], pt[:])
            return [lambda t=t: tr(t) for t in range(4)]

        def q_gen_tasks(u):
            box = {}

            def chunk(c0, u=u, box=box):
                if c0 == 0:
                    box["pq"] = pm_p.tile([128, 512], F32, tag="pm",
                                          name=f"pq{u}")
                pq = box["pq"]
                for c in range(c0, c0 + 2):
                    nc.tensor.matmul(pq[:], lhsT=wq2_sb[:, c, :],
                                     rhs=xqt_sb[:, u, c, :],
                                     start=(c == 0), stop=(c == CB - 1))
                if c0 == CB - 2:
                    nc.vector.tensor_copy(qt2_sb[:, u * 512:(u + 1) * 512],
                                          pq[:])
            return [lambda c0=c0: chunk(c0) for c0 in range(0, CB, 2)]

        def attention_super(u, tasks):
            """Superslot u: 512 queries = slots 2u (cols 0:256) + 2u+1
            (cols 256:512).  Score slots are row-packed pairs:
            paired region r<u: (even-rt block, odd-rt block) full N=512;
            mixed region r=u: A = rt 2u (N=512, masked on cols 0:256),
            B = rt 2u+1 (N=256, slot-2u+1 cols, masked).
            `tasks` are emission thunks (next superslot's kv/q generation,
            loads, previous epilogue) interleaved between score slots to keep
            the PE dense and the HAM clock-gate open."""
            nslots = 4 * u + 4
            slot_idx = [0]

            def run_tasks():
                done = slot_idx[0] + 1
                want = (len(tasks) * done) // nslots
                while task_idx[0] < want:
                    tasks[task_idx[0]]()
                    task_idx[0] += 1
                slot_idx[0] += 1
            task_idx = [0]
            rhs_A = qt2_sb[0:64, u * 512:(u + 1) * 512]
            rhs_B = qt2_sb[64:128, u * 512:(u + 1) * 512]
            rhs_Bh = qt2_sb[64:128, u * 512 + 256:(u + 1) * 512]
            pot = po_p.tile([65, 512], F32, tag="pot", name=f"pot{u}")
            nav = [0]
            n_av_total = 8 * u + 8

            def emit_av(pending):
                for ex_ap, kb, pslice in pending:
                    nc.tensor.matmul(
                        pslice, lhsT=vnat_sb[:, kb, :], rhs=ex_ap,
                        start=(nav[0] == 0), stop=(nav[0] == n_av_total - 1),
                        skip_group_check=True)
                    nav[0] += 1

            pending = []

            def flush_av(keep):
                while len(pending) > keep:
                    emit_av([pending.pop(0)])

            # paired region: rt pairs (2r, 2r+1), r < u — no masks
            for r in range(u):
                for i in range(4):
                    kb_a, kb_b = 8 * r + i, 8 * r + 4 + i
                    ps = ps_p.tile([128, 1024], F32, tag="ps")
                    nc.tensor.matmul(ps[:, 0:512],
                                     lhsT=kvt_sb[0:64, 2 * r, i * 128:(i + 1) * 128],
                                     rhs=rhs_A, start=True, stop=True,
                                     tile_position=(0, 0))
                    nc.tensor.matmul(ps[:, 512:1024],
                                     lhsT=kvt_sb[64:128, 2 * r + 1,
                                                 i * 128:(i + 1) * 128],
                                     rhs=rhs_B, start=True, stop=True,
                                     tile_position=(64, 0))
                    ex = exp_p.tile([128, 1024], BF16, tag="ex")
                    nc.scalar.activation(ex[:], ps[:], EXPF)
                    pending.append((ex[:, 0:512], kb_a, pot[:]))
                    pending.append((ex[:, 512:1024], kb_b, pot[:]))
                    flush_av(2)
                    run_tasks()
            # mixed region: rt 2u (A, boundary for slot 2u) and rt 2u+1
            # (B, boundary for slot 2u+1, N=256)
            for i in range(4):
                kb_a, kb_b = 8 * u + i, 8 * u + 4 + i
                ps = ps_p.tile([128, 1024], F32, tag="ps")
                nc.tensor.matmul(ps[:, 0:512],
                                 lhsT=kvt_sb[0:64, 2 * u, i * 128:(i + 1) * 128],
                                 rhs=rhs_A, start=True, stop=True,
                                 tile_position=(0, 0))
                nc.tensor.matmul(ps[:, 512:768],
                                 lhsT=kvt_sb[64:128, 2 * u + 1,
                                             i * 128:(i + 1) * 128],
                                 rhs=rhs_Bh, start=True, stop=True,
                                 tile_position=(64, 0))
                ex = exp_p.tile([128, 1024], BF16, tag="ex")
                nc.scalar.activation(ex[:, 0:768], ps[:, 0:768], EXPF)
                nc.vector.tensor_mul(ex[:, 0:768], ex[:, 0:768],
                                     mask_sb[:, i, :])
                pending.append((ex[:, 0:512], kb_a, pot[:]))
                pending.append((ex[:, 512:768], kb_b, pot[:, 256:512]))
                flush_av(2)
                run_tasks()
            flush_av(0)
            pot_sb = out_p.tile([65, 512], BF16, tag="pot_sb", name=f"pot_sb{u}")
            nc.vector.tensor_copy(pot_sb[:], pot[:])

            def epilogue(u=u, pot_sb=pot_sb):
                osb = out_p.tile([128, 4, H], BF16, tag="osb", name=f"osb{u}")
                for h in range(4):
                    pt2 = pm_p.tile([128, 65], BF16, tag="pm",
                                    name=f"pt2_{u}{h}")
                    nc.tensor.transpose(pt2[:],
                                        pot_sb[:, h * 128:(h + 1) * 128],
                                        id_sb[:])
                    rcp = out_p.tile([128, 1], F32, tag="rcp")
                    nc.vector.reciprocal(rcp[:], pt2[:, H:H + 1])
                    nc.vector.tensor_scalar_mul(osb[:, h, :], pt2[:, 0:H],
                                                rcp[:])
                nc.sync.dma_start(out=y_d[u], in_=osb[:])
            return epilogue

        # -- emission ------------------------------------------------------
        # Prologue loads: weights first (small, gate kv), xt0/xt1 split for
        # early partial compute, then u=0 q inputs, masks, and gen(1) data.
        nc.sync.dma_start(out=wkva_sb[:], in_=wkva_d.ap())
        load_xt(0, split=True)
        nc.scalar.dma_start(out=wkvb_sb[:], in_=wkvb_d.ap())
        load_xt(1, split=True)
        nc.scalar.dma_start(out=id_sb[:], in_=id_d.ap())
        load_xqt(0)
        nc.scalar.dma_start(out=wq2_sb[:], in_=wq2_d.ap())
        nc.sync.dma_start(out=mask_sb[:], in_=mask_d.ap())
        load_xt(2)
        load_xt(3)
        load_xqt(1)

        # gen(0) inline (nothing to overlap against yet)
        for t in (kv_mm_tasks(0) + kv_mm_tasks(1) + q_gen_tasks(0)
                  + kv_tr_tasks(0) + kv_tr_tasks(1)):
            t()

        epi = None
        for u in range(4):
            tasks = []
            if epi is not None:
                tasks.append(epi)
            if u < 3:
                nu = u + 1
                if 2 * nu + 2 < NRT:
                    tasks.append(lambda rt=2 * nu + 2: load_xt(rt))
                    tasks.append(lambda rt=2 * nu + 3: load_xt(rt))
                if nu + 1 < 4:
                    tasks.append(lambda q=nu + 1: load_xqt(q))
                tasks += kv_mm_tasks(2 * nu)
                tasks += kv_mm_tasks(2 * nu + 1)
                tasks += q_gen_tasks(nu)
                tasks += kv_tr_tasks(2 * nu)
                tasks += kv_tr_tasks(2 * nu + 1)
            epi = attention_super(u, tasks)
        epi()

    _split_sync_waits(nc)
    return nc


def _host_inputs(x, Wq, Wk, Wv):
    """Build the 8 per-core input maps from full fp32 inputs."""
    bf = ml_dtypes.bfloat16
    scale = H ** -0.5
    wkva = np.concatenate([Wk, Wv], axis=1)           # [C, 128]
    wkvb = np.concatenate([Wv, Wk], axis=1)
    wq2 = np.concatenate([Wq * scale, Wq * scale], axis=1)
    ident = np.eye(65, dtype=bf)

    def wtile(w):
        # [C, 128] -> [p, c, w] = [128, CB, 128]
        return np.ascontiguousarray(
            w.reshape(CB, 128, 128).transpose(1, 0, 2)).astype(bf)

    # mask3[p, i, col]: cols 0:256 = A-boundary mask (slot 2u queries),
    # cols 256:512 = ones, cols 512:768 = B-boundary mask (slot 2u+1
    # queries).  Formula for both: allow iff p <= q' + 256j - 128i.
    p = np.arange(128)[:, None, None]
    i_ = np.arange(4)[None, :, None]
    q = np.arange(256)[None, None, :]
    masks = []
    for j in range(2):
        m = (p <= q + 256 * j - 128 * i_)
        m3 = np.concatenate(
            [m, np.ones_like(m), m], axis=2)
        masks.append(np.ascontiguousarray(m3.astype(bf)))

    def retile(a):
        # [C, W] -> [W//512, 128, C//128, 512] (tile, p, c, col)
        w = a.shape[1]
        return np.ascontiguousarray(
            a.reshape(CB, 128, w // 512, 512).transpose(2, 1, 0, 3))

    in_maps = []
    for i in range(NCORES):
        b, j = i // 2, i % 2
        xt = np.ascontiguousarray(x[b].T).astype(bf)
        cols = np.concatenate(
            [np.arange((2 * s + j) * QS, (2 * s + j + 1) * QS)
             for s in range(NSLOT)])
        xqt = xt[:, cols]
        in_maps.append({
            "xt": retile(xt), "xqt": retile(xqt),
            "wkva": wtile(wkva), "wkvb": wtile(wkvb), "wq2": wtile(wq2),
            "mask": masks[j], "ident": ident,
        })
    return in_maps


def _gather(results):
    out = np.empty((B, T, H), np.float32)
    for i in range(NCORES):
        b, j = i // 2, i % 2
        y = np.asarray(results[i]["y"]).astype(np.float32)  # [4, 128, 4, H]
        for u in range(4):
            for h in range(4):
                g = (4 * u + 2 * (h // 2) + j) * 256 + (h % 2) * 128
                out[b, g:g + 128, :] = y[u, :, h, :]
    return out


def _run_sharded(x, Wq, Wk, Wv, trace=False, **kw):
    if "prog" not in _prog_cache:
        _prog_cache["prog"] = _build_program()
    nc = _prog_cache["prog"]
    in_maps = _host_inputs(x, Wq, Wk, Wv)
    res = run_bass_kernel_spmd(nc, in_maps, list(range(NCORES)),
                               trace=trace, **kw)
    return _gather(res.results), res


def kernel(x, Wq, Wk, Wv):
    out, _ = _run_sharded(x, Wq, Wk, Wv, trace=False)
    return out
